# revision 6
# baseline (speedup 1.0000x reference)
"""Trainium2 Bass kernel for nn_RNN_model_GloVe (LSTM + per-step BatchNorm +
locked dropout + temporal max-pool + FC + BCE loss).

Sharding: hidden dim H=1024 split across 8 cores (128 units each). BatchNorm
reduces over the batch axis, which stays whole on every core, so BN is fully
local. The recurrence needs the full hidden state each step, so each step ends
with an AllGather of the per-core h2 slice (h2^T, [128,64] fp32 = 32KB/core).

Self-contained: hardcodes shapes and builds/compiles/runs the Bass program.
"""

import os
import sys

sys.path.insert(0, "/opt/trn_rl_repo")

import numpy as np

import concourse.bacc as bacc
import concourse.bass as bass
import concourse.mybir as mybir
import concourse.tile as tile
from concourse import bass_utils

B = 64
T = 512
D = 300
DP = 384  # D padded to 3*128 (last x k-tile row 383 carries the bias via x=1)
H = 1024
HL = 128  # hidden units per core
GL = 512  # gate columns per core (4 gates x 128)
NCORES = 8
KH = 8  # h k-tiles
KX = 3  # x k-tiles
KT = KH + KX
EPS = 1e-5

F32 = mybir.dt.float32
# float32r: reduced-precision fp32 the PE runs at ~1 col/cycle for N>=256 (vs
# 4 cycles for plain fp32). The BIR verifier requires fp32r matmul inputs to
# be produced by an engine op that rounds, so every matmul operand tile gets
# an on-chip cast-copy. Flip to "float32" if hardware numerics disappoint.
MM_DT = {
    "float32r": mybir.dt.float32r,
    "float32": mybir.dt.float32,
}[os.environ.get("LSTM_MM_DT", "float32r")]
CAST_MM = MM_DT != F32

HT_DMA_SPLIT = int(os.environ.get("LSTM_HT_SPLIT", "8"))
CCIN_DMA_SPLIT = int(os.environ.get("LSTM_CCIN_SPLIT", "2"))

LAST_RESULTS = None  # BassKernelResults of the most recent run (for test.py)


def build_nc(t_steps=T):
    nc = bacc.Bacc(
        "TRN2",
        target_bir_lowering=False,
        debug=False,
        enable_asserts=False,
        num_devices=NCORES,
    )
    t_chunk = t_steps // NCORES
    assert t_chunk * NCORES == t_steps

    # Per-core inputs.
    w_in = nc.dram_tensor("w", [128, KT, GL], F32, kind="ExternalInput")
    xc_in = nc.dram_tensor("xc", [t_chunk * 128, KX * B], F32, kind="ExternalInput")
    gam_in = nc.dram_tensor("gam", [128, 1], F32, kind="ExternalInput")
    bet_in = nc.dram_tensor("bet", [128, 1], F32, kind="ExternalInput")
    ident_in = nc.dram_tensor("ident", [B, B], F32, kind="ExternalInput")
    pooled_out = nc.dram_tensor("pooled", [128, B], F32, kind="ExternalOutput")

    rg = [list(range(NCORES))]

    with tile.TileContext(nc) as tc:
        with (
            tc.tile_pool(name="const", bufs=1) as cpool,
            tc.tile_pool(name="xin", bufs=6) as xpool,
            tc.tile_pool(name="hin", bufs=3) as hpool,
            tc.tile_pool(name="work", bufs=3) as wpool,
            tc.tile_pool(name="bn", bufs=3) as bnpool,
            tc.tile_pool(name="psg", bufs=2, space="PSUM") as ppool,
            tc.tile_pool(name="pst", bufs=2, space="PSUM") as ptpool,
            tc.tile_pool(name="dram", bufs=2, space="DRAM") as dpool,
            tc.tile_pool(name="dramx", bufs=1, space="DRAM") as dxpool,
        ):
            # ---- one-time setup ----
            # Gather the full (pre-shuffled) x across cores: each core uploads
            # only its T/8 chunk; the AllGather concatenation reproduces the
            # t-major [T*128, KX*B] layout exactly.
            x_stage = dxpool.tile([t_chunk * 128, KX * B], F32)
            for s in range(4):
                r = t_chunk * 128 // 4
                nc.sync.dma_start(
                    out=x_stage[s * r : (s + 1) * r, :],
                    in_=xc_in[s * r : (s + 1) * r, :],
                )
            xr_full = dxpool.tile(
                [t_steps * 128, KX * B], F32, addr_space="Shared"
            )
            nc.gpsimd.collective_compute(
                "AllGather",
                mybir.AluOpType.bypass,
                replica_groups=rg,
                ins=[x_stage[:]],
                outs=[xr_full[:]],
            )

            w_sb = cpool.tile([128, KT, GL], F32)
            nc.sync.dma_start(out=w_sb[:], in_=w_in[:])
            if CAST_MM:
                w_r = cpool.tile([128, KT, GL], MM_DT)
                nc.vector.tensor_copy(w_r[:], w_sb[:])
            else:
                w_r = w_sb
            gam_sb = cpool.tile([128, 1], F32)
            nc.sync.dma_start(out=gam_sb[:], in_=gam_in[:])
            bet_sb = cpool.tile([128, 1], F32)
            nc.sync.dma_start(out=bet_sb[:], in_=bet_in[:])
            ident_sb = cpool.tile([B, B], F32)
            nc.sync.dma_start(out=ident_sb[:], in_=ident_in[:])
            eps_sb = cpool.tile([128, 1], F32)
            nc.vector.memset(eps_sb[:], EPS)

            c_sb = cpool.tile([B, HL], F32)
            nc.vector.memset(c_sb[:], 0.0)
            pooled_sb = cpool.tile([128, B], F32)
            nc.vector.memset(pooled_sb[:], -3.0e38)

            hT_cur = None  # SBUF [128, KH, B] holding full h^T k-tiles

            for t in range(t_steps):
                x_sb = xpool.tile([128, KX, B], F32)
                nc.sync.dma_start(
                    out=x_sb[:],
                    in_=xr_full[t * 128 : (t + 1) * 128, :].rearrange(
                        "p (k b) -> p k b", k=KX
                    ),
                )
                if CAST_MM:
                    x_r = xpool.tile([128, KX, B], MM_DT)
                    nc.vector.tensor_copy(x_r[:], x_sb[:])
                else:
                    x_r = x_sb

                psum_g = ppool.tile([B, GL], F32)
                # x-projection (and bias, folded into x row 383 == 1): these
                # only depend on x, so they fill the PE while the AllGather of
                # the previous step is still in flight.
                for kk in range(KX):
                    nc.tensor.matmul(
                        psum_g[:],
                        lhsT=x_r[:, kk, :],
                        rhs=w_r[:, KH + kk, :],
                        start=(kk == 0),
                        stop=(kk == KX - 1 and t == 0),
                    )
                if t > 0:
                    for kk in range(KH):
                        nc.tensor.matmul(
                            psum_g[:],
                            lhsT=hT_cur[:, kk, :],
                            rhs=w_r[:, kk, :],
                            start=False,
                            stop=(kk == KH - 1),
                        )

                # Gate order along GL: [i, f, o, g].
                sig_ifo = wpool.tile([B, 3 * HL], F32)
                nc.scalar.activation(
                    sig_ifo[:], psum_g[:, 0 : 3 * HL],
                    mybir.ActivationFunctionType.Sigmoid,
                )
                tanh_g = wpool.tile([B, HL], F32)
                nc.scalar.activation(
                    tanh_g[:], psum_g[:, 3 * HL : 4 * HL],
                    mybir.ActivationFunctionType.Tanh,
                )

                t1 = wpool.tile([B, HL], F32)
                nc.vector.tensor_mul(t1[:], sig_ifo[:, HL : 2 * HL], c_sb[:])
                t2 = wpool.tile([B, HL], F32)
                nc.vector.tensor_mul(t2[:], sig_ifo[:, 0:HL], tanh_g[:])
                nc.vector.tensor_add(c_sb[:], t1[:], t2[:])
                tanh_c = wpool.tile([B, HL], F32)
                nc.scalar.activation(
                    tanh_c[:], c_sb[:], mybir.ActivationFunctionType.Tanh
                )
                h2_sb = wpool.tile([B, HL], F32)
                nc.vector.tensor_mul(h2_sb[:], sig_ifo[:, 2 * HL : 3 * HL], tanh_c[:])

                # h2^T for the AllGather (and for BN, whose batch axis must be
                # the free axis).
                pT = ptpool.tile([HL, B], F32)
                nc.tensor.transpose(pT[:], h2_sb[:], ident_sb[:])
                h2T = wpool.tile([HL, B], F32)
                nc.vector.tensor_copy(h2T[:], pT[:])

                if t < t_steps - 1:
                    if CAST_MM:
                        h2T_r = wpool.tile([HL, B], MM_DT)
                        nc.vector.tensor_copy(h2T_r[:], pT[:])
                    else:
                        h2T_r = h2T
                    cc_in = dpool.tile([HL, B], MM_DT, tag="cc_in")
                    r = HL // CCIN_DMA_SPLIT
                    for s in range(CCIN_DMA_SPLIT):
                        nc.sync.dma_start(
                            out=cc_in[s * r : (s + 1) * r, :],
                            in_=h2T_r[s * r : (s + 1) * r, :],
                        )
                    cc_out = dpool.tile(
                        [NCORES * HL, B], MM_DT, tag="cc_out", addr_space="Shared"
                    )
                    nc.gpsimd.collective_compute(
                        "AllGather",
                        mybir.AluOpType.bypass,
                        replica_groups=rg,
                        ins=[cc_in[:]],
                        outs=[cc_out[:]],
                    )
                    hT_next = hpool.tile([128, KH, B], MM_DT)
                    step = KH // HT_DMA_SPLIT
                    for s in range(HT_DMA_SPLIT):
                        nc.sync.dma_start(
                            out=hT_next[:, s * step : (s + 1) * step, :],
                            in_=cc_out[
                                s * step * 128 : (s + 1) * step * 128, :
                            ].rearrange("(k p) b -> p k b", p=128),
                        )
                    hT_cur = hT_next

                # BatchNorm (train mode, biased var over the full batch) +
                # temporal max-pool — off the critical path.
                stats6 = bnpool.tile([HL, 6], F32)
                nc.vector.bn_stats(stats6[:], h2T[:])
                mv = bnpool.tile([HL, 2], F32)
                nc.vector.bn_aggr(mv[:], stats6[:])
                std = bnpool.tile([HL, 1], F32)
                nc.scalar.activation(
                    std[:], mv[:, 1:2],
                    mybir.ActivationFunctionType.Sqrt,
                    bias=eps_sb[:],
                )
                rinv = bnpool.tile([HL, 1], F32)
                nc.vector.reciprocal(rinv[:], std[:])
                scl = bnpool.tile([HL, 1], F32)
                nc.vector.tensor_mul(scl[:], rinv[:], gam_sb[:])
                msc = bnpool.tile([HL, 1], F32)
                nc.vector.tensor_mul(msc[:], mv[:, 0:1], scl[:])
                shf = bnpool.tile([HL, 1], F32)
                nc.vector.tensor_sub(shf[:], bet_sb[:], msc[:])
                hn = bnpool.tile([HL, B], F32)
                nc.vector.tensor_scalar(
                    hn[:], h2T[:], scl[:], shf[:],
                    op0=mybir.AluOpType.mult, op1=mybir.AluOpType.add,
                )
                nc.vector.tensor_max(pooled_sb[:], pooled_sb[:], hn[:])

            nc.sync.dma_start(out=pooled_out[:], in_=pooled_sb[:])

    nc.compile()
    return nc


def prep_inputs(x, W_ih, W_hh, b_ih, b_hh, gamma, beta, t_steps=T):
    """Build the 8 per-core input maps (numpy only)."""
    x = np.asarray(x, np.float32)
    W_ih = np.asarray(W_ih, np.float32)
    W_hh = np.asarray(W_hh, np.float32)
    bias = np.asarray(b_ih, np.float32) + np.asarray(b_hh, np.float32)
    gamma = np.asarray(gamma, np.float32)
    beta = np.asarray(beta, np.float32)
    b_sz, t_sz = x.shape[0], x.shape[1]
    t_chunk = t_steps // NCORES

    # x pre-shuffle: xr[t, p, kk, b] = x[b, t, kk*128+p], row 383 = 1.0 (bias).
    xpad = np.zeros((t_sz, DP, b_sz), np.float32)
    xpad[:, :D, :] = x.transpose(1, 2, 0)
    xpad[:, DP - 1, :] = 1.0
    xr = np.ascontiguousarray(
        xpad.reshape(t_sz, KX, 128, b_sz).transpose(0, 2, 1, 3)
    ).reshape(t_sz, 128, KX * b_sz)

    ident = np.eye(b_sz, dtype=np.float32)
    in_maps = []
    for k in range(NCORES):
        idx = np.arange(k * HL, (k + 1) * HL)
        gate_rows = np.concatenate([g * H + idx for g in (0, 1, 3, 2)])  # i,f,o,g
        wT = np.zeros((KT * 128, GL), np.float32)
        wT[:H, :] = W_hh[gate_rows, :].T
        wT[H : H + D, :] = W_ih[gate_rows, :].T
        wT[H + DP - 1, :] = bias[gate_rows]
        w_tiles = np.ascontiguousarray(wT.reshape(KT, 128, GL).transpose(1, 0, 2))
        in_maps.append(
            {
                "w": w_tiles,
                "xc": np.ascontiguousarray(
                    xr[k * t_chunk : (k + 1) * t_chunk]
                ).reshape(t_chunk * 128, KX * b_sz),
                "gam": np.ascontiguousarray(gamma[idx][:, None]),
                "bet": np.ascontiguousarray(beta[idx][:, None]),
                "ident": ident,
            }
        )
    return in_maps


_NC_CACHE = {}


def run_cores(in_maps, t_steps=T, trace=False):
    global LAST_RESULTS
    if t_steps not in _NC_CACHE:
        _NC_CACHE[t_steps] = build_nc(t_steps)
    nc = _NC_CACHE[t_steps]
    res = bass_utils.run_bass_kernel_spmd(
        nc, in_maps, core_ids=list(range(NCORES)), trace=trace
    )
    LAST_RESULTS = res
    return res


def kernel(x, t, mask, W_ih, W_hh, b_ih, b_hh, gamma, beta, fc_w, fc_b):
    in_maps = prep_inputs(x, W_ih, W_hh, b_ih, b_hh, gamma, beta)
    trace = os.environ.get("LSTM_TRACE", "0") == "1"
    res = run_cores(in_maps, T, trace=trace)

    pooledT = np.stack([res.results[k]["pooled"] for k in range(NCORES)])
    pooled = pooledT.transpose(2, 0, 1).reshape(B, H)  # [b, core*128+p]
    pooled = pooled * np.asarray(mask, np.float32)
    z = pooled @ np.asarray(fc_w, np.float32)[0] + np.asarray(fc_b, np.float32)[0]
    tt = np.asarray(t, np.float32)
    loss = np.float32(np.mean(np.logaddexp(np.float32(0.0), z) - z * tt))
    return (loss, z.astype(np.float32))


# revision 14
# speedup vs baseline: 1.0249x; 1.0249x over previous
"""Trainium2 Bass kernel for nn_RNN_model_GloVe (LSTM + per-step BatchNorm +
locked dropout + temporal max-pool + FC + BCE loss).

Sharding: hidden dim H=1024 split across 8 cores (128 units each). BatchNorm
reduces over the batch axis, which stays whole on every core, so BN is fully
local. The recurrence needs the full hidden state each step, so each step ends
with an AllGather of the per-core h2 slice (h2^T, [128,64] fp32 = 32KB/core).

Self-contained: hardcodes shapes and builds/compiles/runs the Bass program.
"""

import os
import sys

sys.path.insert(0, "/opt/trn_rl_repo")

import numpy as np

import concourse.bacc as bacc
import concourse.bass as bass
import concourse.mybir as mybir
import concourse.tile as tile
from concourse import bass_utils
from concourse.tile import add_dep_helper

B = 64
T = 512
D = 300
DP = 384  # D padded to 3*128 (last x k-tile row 383 carries the bias via x=1)
H = 1024
HL = 128  # hidden units per core
GL = 512  # gate columns per core (4 gates x 128)
NCORES = 8
KH = 8  # h k-tiles
KX = 3  # x k-tiles
KT = KH + KX
EPS = 1e-5

F32 = mybir.dt.float32
# float32r: reduced-precision fp32 the PE runs at ~1 col/cycle for N>=256 (vs
# 4 cycles for plain fp32). The BIR verifier requires fp32r matmul inputs to
# be produced by an engine op that rounds, so every matmul operand tile gets
# an on-chip cast-copy. Flip to "float32" if hardware numerics disappoint.
MM_DT = {
    "float32r": mybir.dt.float32r,
    "float32": mybir.dt.float32,
}[os.environ.get("LSTM_MM_DT", "float32r")]
CAST_MM = MM_DT != F32

HT_DMA_SPLIT = int(os.environ.get("LSTM_HT_SPLIT", "8"))
CCIN_DMA_SPLIT = int(os.environ.get("LSTM_CCIN_SPLIT", "1"))
N_WARM = int(os.environ.get("LSTM_WARM", "8"))

LAST_RESULTS = None  # BassKernelResults of the most recent run (for test.py)


def build_nc(t_steps=T):
    nc = bacc.Bacc(
        "TRN2",
        target_bir_lowering=False,
        debug=False,
        enable_asserts=False,
        num_devices=NCORES,
    )
    t_chunk = t_steps // NCORES
    assert t_chunk * NCORES == t_steps

    # Per-core inputs.
    w_in = nc.dram_tensor("w", [128, KT, GL], F32, kind="ExternalInput")
    xc_in = nc.dram_tensor("xc", [t_chunk * 128, KX * B], F32, kind="ExternalInput")
    gam_in = nc.dram_tensor("gam", [128, 1], F32, kind="ExternalInput")
    bet_in = nc.dram_tensor("bet", [128, 1], F32, kind="ExternalInput")
    ident_in = nc.dram_tensor("ident", [B, B], F32, kind="ExternalInput")
    pooled_out = nc.dram_tensor("pooled", [128, B], F32, kind="ExternalOutput")

    rg = [list(range(NCORES))]

    with tile.TileContext(nc) as tc:
        with (
            tc.tile_pool(name="const", bufs=1) as cpool,
            tc.tile_pool(name="xin", bufs=6) as xpool,
            tc.tile_pool(name="hin", bufs=3) as hpool,
            tc.tile_pool(name="work", bufs=3) as wpool,
            tc.tile_pool(name="bn", bufs=3) as bnpool,
            tc.tile_pool(name="psg", bufs=2, space="PSUM") as ppool,
            tc.tile_pool(name="pst", bufs=2, space="PSUM") as ptpool,
            tc.tile_pool(name="warm", bufs=1, space="PSUM") as wmpool,
            tc.tile_pool(name="dram", bufs=2, space="DRAM") as dpool,
            tc.tile_pool(name="dramx", bufs=1, space="DRAM") as dxpool,
        ):
            # ---- one-time setup ----
            # Gather the full (pre-shuffled) x across cores: each core uploads
            # only its T/8 chunk; the AllGather concatenation reproduces the
            # t-major [T*128, KX*B] layout exactly.
            x_stage = dxpool.tile([t_chunk * 128, KX * B], F32)
            for s in range(4):
                r = t_chunk * 128 // 4
                nc.sync.dma_start(
                    out=x_stage[s * r : (s + 1) * r, :],
                    in_=xc_in[s * r : (s + 1) * r, :],
                )
            xr_full = dxpool.tile(
                [t_steps * 128, KX * B], F32, addr_space="Shared"
            )
            nc.gpsimd.collective_compute(
                "AllGather",
                mybir.AluOpType.bypass,
                replica_groups=rg,
                ins=[x_stage[:]],
                outs=[xr_full[:]],
            )

            w_sb = cpool.tile([128, KT, GL], F32)
            nc.sync.dma_start(out=w_sb[:], in_=w_in[:])
            if CAST_MM:
                w_r = cpool.tile([128, KT, GL], MM_DT)
                nc.vector.tensor_copy(w_r[:], w_sb[:])
            else:
                w_r = w_sb
            gam_sb = cpool.tile([128, 1], F32)
            nc.sync.dma_start(out=gam_sb[:], in_=gam_in[:])
            bet_sb = cpool.tile([128, 1], F32)
            nc.sync.dma_start(out=bet_sb[:], in_=bet_in[:])
            ident_f = cpool.tile([B, B], F32)
            nc.sync.dma_start(out=ident_f[:], in_=ident_in[:])
            if CAST_MM:
                ident_sb = cpool.tile([B, B], MM_DT)
                nc.vector.tensor_copy(ident_sb[:], ident_f[:])
            else:
                ident_sb = ident_f
            eps_sb = cpool.tile([128, 1], F32)
            nc.vector.memset(eps_sb[:], EPS)

            c_sb = cpool.tile([B, HL], F32)
            nc.vector.memset(c_sb[:], 0.0)
            pooled_sb = cpool.tile([128, B], F32)
            nc.vector.memset(pooled_sb[:], -3.0e38)

            hT_cur = None  # SBUF [128, KH, B] holding full h^T k-tiles

            for t in range(t_steps):
                x_sb = xpool.tile([128, KX, B], F32)
                nc.sync.dma_start(
                    out=x_sb[:],
                    in_=xr_full[t * 128 : (t + 1) * 128, :].rearrange(
                        "p (k b) -> p k b", k=KX
                    ),
                )
                if CAST_MM:
                    x_r = xpool.tile([128, KX, B], MM_DT)
                    nc.vector.tensor_copy(x_r[:], x_sb[:])
                else:
                    x_r = x_sb

                psum_g = ppool.tile([B, GL], F32)
                # x-projection (and bias, folded into x row 383 == 1): these
                # only depend on x, so they fill the PE while the AllGather of
                # the previous step is still in flight.
                for kk in range(KX):
                    nc.tensor.matmul(
                        psum_g[:],
                        lhsT=x_r[:, kk, :],
                        rhs=w_r[:, KH + kk, :],
                        start=(kk == 0),
                        stop=(kk == KX - 1 and t == 0),
                    )
                if t > 0:
                    for kk in range(KH):
                        nc.tensor.matmul(
                            psum_g[:],
                            lhsT=hT_cur[:, kk, :],
                            rhs=w_r[:, kk, :],
                            start=False,
                            stop=(kk == KH - 1),
                        )

                # Gate order along GL: [i, f, o, g]. Split the sigmoid so the
                # c-update multiplies can start before sigma(o) is done.
                sig_if = wpool.tile([B, 2 * HL], F32)
                nc.scalar.activation(
                    sig_if[:], psum_g[:, 0 : 2 * HL],
                    mybir.ActivationFunctionType.Sigmoid,
                )
                tanh_g = wpool.tile([B, HL], F32)
                nc.scalar.activation(
                    tanh_g[:], psum_g[:, 3 * HL : 4 * HL],
                    mybir.ActivationFunctionType.Tanh,
                )
                sig_o = wpool.tile([B, HL], F32)
                nc.scalar.activation(
                    sig_o[:], psum_g[:, 2 * HL : 3 * HL],
                    mybir.ActivationFunctionType.Sigmoid,
                )

                t1 = wpool.tile([B, HL], F32)
                nc.vector.tensor_mul(t1[:], sig_if[:, HL : 2 * HL], c_sb[:])
                t2 = wpool.tile([B, HL], F32)
                nc.vector.tensor_mul(t2[:], sig_if[:, 0:HL], tanh_g[:])
                nc.vector.tensor_add(c_sb[:], t1[:], t2[:])
                tanh_c = wpool.tile([B, HL], F32)
                nc.scalar.activation(
                    tanh_c[:], c_sb[:], mybir.ActivationFunctionType.Tanh
                )
                h2_sb = wpool.tile([B, HL], MM_DT)
                nc.vector.tensor_mul(h2_sb[:], sig_o[:], tanh_c[:])

                # h2^T for the AllGather (and for BN, whose batch axis must be
                # the free axis). Transpose rounds straight to fp32r in PSUM
                # so the comm payload can be DMA'd from PSUM with no extra
                # cast on the critical path.
                pT = ptpool.tile([HL, B], MM_DT)
                nc.tensor.transpose(pT[:], h2_sb[:], ident_sb[:])
                h2T = wpool.tile([HL, B], MM_DT)
                nc.vector.tensor_copy(h2T[:], pT[:])

                if t < t_steps - 1:
                    cc_in = dpool.tile([HL, B], MM_DT, tag="cc_in")
                    r = HL // CCIN_DMA_SPLIT
                    cc_dma = None
                    for s in range(CCIN_DMA_SPLIT):
                        cc_dma = nc.gpsimd.dma_start(
                            out=cc_in[s * r : (s + 1) * r, :],
                            in_=h2T[s * r : (s + 1) * r, :],
                        )
                    cc_out = dpool.tile(
                        [NCORES * HL, B], MM_DT, tag="cc_out", addr_space="Shared"
                    )
                    nc.gpsimd.collective_compute(
                        "AllGather",
                        mybir.AluOpType.bypass,
                        replica_groups=rg,
                        ins=[cc_in[:]],
                        outs=[cc_out[:]],
                    )
                    # Keep the PE's activity monitor from re-throttling the
                    # clock during the AllGather gap: a chain of tiny matmuls
                    # dep-chained behind the comm-payload DMA.
                    if N_WARM:
                        warm_ps = wmpool.tile([1, GL], F32, tag="warm")
                        first = None
                        for _ in range(N_WARM):
                            wmm = nc.tensor.matmul(
                                warm_ps[:],
                                lhsT=x_r[:, 0, 0:1],
                                rhs=w_r[:, 0, :],
                                start=True,
                                stop=True,
                            )
                            if first is None and cc_dma is not None:
                                add_dep_helper(
                                    wmm.ins, cc_dma.ins, sync=True,
                                    reason="PE warmer waits for comm DMA",
                                )
                                first = wmm
                    hT_next = hpool.tile([128, KH, B], MM_DT)
                    step = KH // HT_DMA_SPLIT
                    qeng = [nc.sync, nc.scalar, nc.gpsimd, nc.sync,
                            nc.scalar, nc.gpsimd, nc.sync, nc.scalar]
                    for s in range(HT_DMA_SPLIT):
                        qeng[s % len(qeng)].dma_start(
                            out=hT_next[:, s * step : (s + 1) * step, :],
                            in_=cc_out[
                                s * step * 128 : (s + 1) * step * 128, :
                            ].rearrange("(k p) b -> p k b", p=128),
                        )
                    hT_cur = hT_next

                # BatchNorm (train mode, biased var over the full batch) +
                # temporal max-pool — off the critical path.
                stats6 = bnpool.tile([HL, 6], F32)
                nc.vector.bn_stats(stats6[:], h2T[:])
                mv = bnpool.tile([HL, 2], F32)
                nc.vector.bn_aggr(mv[:], stats6[:])
                std = bnpool.tile([HL, 1], F32)
                nc.scalar.activation(
                    std[:], mv[:, 1:2],
                    mybir.ActivationFunctionType.Sqrt,
                    bias=eps_sb[:],
                )
                rinv = bnpool.tile([HL, 1], F32)
                nc.vector.reciprocal(rinv[:], std[:])
                scl = bnpool.tile([HL, 1], F32)
                nc.vector.tensor_mul(scl[:], rinv[:], gam_sb[:])
                msc = bnpool.tile([HL, 1], F32)
                nc.vector.tensor_mul(msc[:], mv[:, 0:1], scl[:])
                shf = bnpool.tile([HL, 1], F32)
                nc.vector.tensor_sub(shf[:], bet_sb[:], msc[:])
                hn = bnpool.tile([HL, B], F32)
                nc.vector.tensor_scalar(
                    hn[:], h2T[:], scl[:], shf[:],
                    op0=mybir.AluOpType.mult, op1=mybir.AluOpType.add,
                )
                nc.vector.tensor_max(pooled_sb[:], pooled_sb[:], hn[:])

            nc.sync.dma_start(out=pooled_out[:], in_=pooled_sb[:])

    nc.compile()
    return nc


def prep_inputs(x, W_ih, W_hh, b_ih, b_hh, gamma, beta, t_steps=T):
    """Build the 8 per-core input maps (numpy only)."""
    x = np.asarray(x, np.float32)
    W_ih = np.asarray(W_ih, np.float32)
    W_hh = np.asarray(W_hh, np.float32)
    bias = np.asarray(b_ih, np.float32) + np.asarray(b_hh, np.float32)
    gamma = np.asarray(gamma, np.float32)
    beta = np.asarray(beta, np.float32)
    b_sz, t_sz = x.shape[0], x.shape[1]
    t_chunk = t_steps // NCORES

    # x pre-shuffle: xr[t, p, kk, b] = x[b, t, kk*128+p], row 383 = 1.0 (bias).
    xpad = np.zeros((t_sz, DP, b_sz), np.float32)
    xpad[:, :D, :] = x.transpose(1, 2, 0)
    xpad[:, DP - 1, :] = 1.0
    xr = np.ascontiguousarray(
        xpad.reshape(t_sz, KX, 128, b_sz).transpose(0, 2, 1, 3)
    ).reshape(t_sz, 128, KX * b_sz)

    ident = np.eye(b_sz, dtype=np.float32)
    in_maps = []
    for k in range(NCORES):
        idx = np.arange(k * HL, (k + 1) * HL)
        gate_rows = np.concatenate([g * H + idx for g in (0, 1, 3, 2)])  # i,f,o,g
        wT = np.zeros((KT * 128, GL), np.float32)
        wT[:H, :] = W_hh[gate_rows, :].T
        wT[H : H + D, :] = W_ih[gate_rows, :].T
        wT[H + DP - 1, :] = bias[gate_rows]
        w_tiles = np.ascontiguousarray(wT.reshape(KT, 128, GL).transpose(1, 0, 2))
        in_maps.append(
            {
                "w": w_tiles,
                "xc": np.ascontiguousarray(
                    xr[k * t_chunk : (k + 1) * t_chunk]
                ).reshape(t_chunk * 128, KX * b_sz),
                "gam": np.ascontiguousarray(gamma[idx][:, None]),
                "bet": np.ascontiguousarray(beta[idx][:, None]),
                "ident": ident,
            }
        )
    return in_maps


_NC_CACHE = {}


def run_cores(in_maps, t_steps=T, trace=False):
    global LAST_RESULTS
    if t_steps not in _NC_CACHE:
        _NC_CACHE[t_steps] = build_nc(t_steps)
    nc = _NC_CACHE[t_steps]
    res = bass_utils.run_bass_kernel_spmd(
        nc, in_maps, core_ids=list(range(NCORES)), trace=trace
    )
    LAST_RESULTS = res
    return res


def kernel(x, t, mask, W_ih, W_hh, b_ih, b_hh, gamma, beta, fc_w, fc_b):
    in_maps = prep_inputs(x, W_ih, W_hh, b_ih, b_hh, gamma, beta)
    trace = os.environ.get("LSTM_TRACE", "0") == "1"
    res = run_cores(in_maps, T, trace=trace)

    pooledT = np.stack([res.results[k]["pooled"] for k in range(NCORES)])
    pooled = pooledT.transpose(2, 0, 1).reshape(B, H)  # [b, core*128+p]
    pooled = pooled * np.asarray(mask, np.float32)
    z = pooled @ np.asarray(fc_w, np.float32)[0] + np.asarray(fc_b, np.float32)[0]
    tt = np.asarray(t, np.float32)
    loss = np.float32(np.mean(np.logaddexp(np.float32(0.0), z) - z * tt))
    return (loss, z.astype(np.float32))


# revision 17
# speedup vs baseline: 1.1234x; 1.0961x over previous
"""Trainium2 Bass kernel for nn_RNN_model_GloVe (LSTM + per-step BatchNorm +
locked dropout + temporal max-pool + FC + BCE loss).

Sharding: hidden dim H=1024 split across 8 cores (128 units each). BatchNorm
reduces over the batch axis, which stays whole on every core, so BN is fully
local. The recurrence needs the full hidden state each step, so each step ends
with an AllGather of the per-core h2 slice (h2^T, [128,64] fp32 = 32KB/core).

Self-contained: hardcodes shapes and builds/compiles/runs the Bass program.
"""

import os
import sys

sys.path.insert(0, "/opt/trn_rl_repo")

import numpy as np

import concourse.bacc as bacc
import concourse.bass as bass
import concourse.mybir as mybir
import concourse.tile as tile
from concourse import bass_utils
from concourse.tile import add_dep_helper

B = 64
T = 512
D = 300
DP = 384  # D padded to 3*128 (last x k-tile row 383 carries the bias via x=1)
H = 1024
HL = 128  # hidden units per core
GL = 512  # gate columns per core (4 gates x 128)
NCORES = 8
KH = 8  # h k-tiles
KX = 3  # x k-tiles
KT = KH + KX
EPS = 1e-5

F32 = mybir.dt.float32
# float32r: reduced-precision fp32 the PE runs at ~1 col/cycle for N>=256 (vs
# 4 cycles for plain fp32). The BIR verifier requires fp32r matmul inputs to
# be produced by an engine op that rounds, so every matmul operand tile gets
# an on-chip cast-copy. Flip to "float32" if hardware numerics disappoint.
MM_DT = {
    "float32r": mybir.dt.float32r,
    "float32": mybir.dt.float32,
    "bfloat16": mybir.dt.bfloat16,
}[os.environ.get("LSTM_MM_DT", "float32r")]
CAST_MM = MM_DT != F32

HT_DMA_SPLIT = int(os.environ.get("LSTM_HT_SPLIT", "8"))
CCIN_DMA_SPLIT = int(os.environ.get("LSTM_CCIN_SPLIT", "1"))
N_WARM = int(os.environ.get("LSTM_WARM", "8"))

LAST_RESULTS = None  # BassKernelResults of the most recent run (for test.py)


def build_nc(t_steps=T):
    nc = bacc.Bacc(
        "TRN2",
        target_bir_lowering=False,
        debug=False,
        enable_asserts=False,
        num_devices=NCORES,
    )
    t_chunk = t_steps // NCORES
    assert t_chunk * NCORES == t_steps

    # Per-core inputs.
    w_in = nc.dram_tensor("w", [128, KT, GL], F32, kind="ExternalInput")
    xc_in = nc.dram_tensor("xc", [t_chunk * 128, KX * B], F32, kind="ExternalInput")
    gam_in = nc.dram_tensor("gam", [128, 1], F32, kind="ExternalInput")
    bet_in = nc.dram_tensor("bet", [128, 1], F32, kind="ExternalInput")
    ident_in = nc.dram_tensor("ident", [B, B], F32, kind="ExternalInput")
    pooled_out = nc.dram_tensor("pooled", [128, B], F32, kind="ExternalOutput")

    rg = [list(range(NCORES))]

    with tile.TileContext(nc) as tc:
        with (
            tc.tile_pool(name="const", bufs=1) as cpool,
            tc.tile_pool(name="xin", bufs=6) as xpool,
            tc.tile_pool(name="hin", bufs=3) as hpool,
            tc.tile_pool(name="work", bufs=3) as wpool,
            tc.tile_pool(name="bn", bufs=3) as bnpool,
            tc.tile_pool(name="psg", bufs=2, space="PSUM") as ppool,
            tc.tile_pool(name="pst", bufs=2, space="PSUM") as ptpool,
            tc.tile_pool(name="warm", bufs=1, space="PSUM") as wmpool,
            tc.tile_pool(name="dram", bufs=2, space="DRAM") as dpool,
            tc.tile_pool(name="dramx", bufs=1, space="DRAM") as dxpool,
        ):
            # ---- one-time setup ----
            # Gather the full (pre-shuffled) x across cores: each core uploads
            # only its T/8 chunk; the AllGather concatenation reproduces the
            # t-major [T*128, KX*B] layout exactly.
            x_stage = dxpool.tile([t_chunk * 128, KX * B], F32)
            for s in range(4):
                r = t_chunk * 128 // 4
                nc.sync.dma_start(
                    out=x_stage[s * r : (s + 1) * r, :],
                    in_=xc_in[s * r : (s + 1) * r, :],
                )
            xr_full = dxpool.tile(
                [t_steps * 128, KX * B], F32, addr_space="Shared"
            )
            nc.gpsimd.collective_compute(
                "AllGather",
                mybir.AluOpType.bypass,
                replica_groups=rg,
                ins=[x_stage[:]],
                outs=[xr_full[:]],
            )

            w_sb = cpool.tile([128, KT, GL], F32)
            nc.sync.dma_start(out=w_sb[:], in_=w_in[:])
            if CAST_MM:
                w_r = cpool.tile([128, KT, GL], MM_DT)
                nc.vector.tensor_copy(w_r[:], w_sb[:])
            else:
                w_r = w_sb
            gam_sb = cpool.tile([128, 1], F32)
            nc.sync.dma_start(out=gam_sb[:], in_=gam_in[:])
            bet_sb = cpool.tile([128, 1], F32)
            nc.sync.dma_start(out=bet_sb[:], in_=bet_in[:])
            ident_f = cpool.tile([B, B], F32)
            nc.sync.dma_start(out=ident_f[:], in_=ident_in[:])
            if CAST_MM:
                ident_sb = cpool.tile([B, B], MM_DT)
                nc.vector.tensor_copy(ident_sb[:], ident_f[:])
            else:
                ident_sb = ident_f
            eps_sb = cpool.tile([128, 1], F32)
            nc.vector.memset(eps_sb[:], EPS)

            c_sb = cpool.tile([B, HL], F32)
            nc.vector.memset(c_sb[:], 0.0)
            pooled_sb = cpool.tile([128, B], F32)
            nc.vector.memset(pooled_sb[:], -3.0e38)

            hT_cur = None  # SBUF [128, KH, B] holding full h^T k-tiles

            for t in range(t_steps):
                x_sb = xpool.tile([128, KX, B], F32)
                nc.sync.dma_start(
                    out=x_sb[:],
                    in_=xr_full[t * 128 : (t + 1) * 128, :].rearrange(
                        "p (k b) -> p k b", k=KX
                    ),
                )
                if CAST_MM:
                    x_r = xpool.tile([128, KX, B], MM_DT)
                    nc.vector.tensor_copy(x_r[:], x_sb[:])
                else:
                    x_r = x_sb

                psum_g = ppool.tile([B, GL], F32)
                # x-projection (and bias, folded into x row 383 == 1): these
                # only depend on x, so they fill the PE while the AllGather of
                # the previous step is still in flight.
                for kk in range(KX):
                    nc.tensor.matmul(
                        psum_g[:],
                        lhsT=x_r[:, kk, :],
                        rhs=w_r[:, KH + kk, :],
                        start=(kk == 0),
                        stop=(kk == KX - 1 and t == 0),
                    )
                if t > 0:
                    for kk in range(KH):
                        nc.tensor.matmul(
                            psum_g[:],
                            lhsT=hT_cur[:, kk, :],
                            rhs=w_r[:, kk, :],
                            start=False,
                            stop=(kk == KH - 1),
                        )

                # Gate order along GL: [i, f, o, g]. Split the sigmoid so the
                # c-update multiplies can start before sigma(o) is done.
                sig_if = wpool.tile([B, 2 * HL], F32)
                nc.scalar.activation(
                    sig_if[:], psum_g[:, 0 : 2 * HL],
                    mybir.ActivationFunctionType.Sigmoid,
                )
                tanh_g = wpool.tile([B, HL], F32)
                nc.scalar.activation(
                    tanh_g[:], psum_g[:, 3 * HL : 4 * HL],
                    mybir.ActivationFunctionType.Tanh,
                )
                sig_o = wpool.tile([B, HL], F32)
                nc.scalar.activation(
                    sig_o[:], psum_g[:, 2 * HL : 3 * HL],
                    mybir.ActivationFunctionType.Sigmoid,
                )

                t1 = wpool.tile([B, HL], F32)
                nc.vector.tensor_mul(t1[:], sig_if[:, HL : 2 * HL], c_sb[:])
                t2 = wpool.tile([B, HL], F32)
                nc.vector.tensor_mul(t2[:], sig_if[:, 0:HL], tanh_g[:])
                nc.vector.tensor_add(c_sb[:], t1[:], t2[:])
                tanh_c = wpool.tile([B, HL], F32)
                nc.scalar.activation(
                    tanh_c[:], c_sb[:], mybir.ActivationFunctionType.Tanh
                )
                h2_sb = wpool.tile([B, HL], MM_DT)
                nc.vector.tensor_mul(h2_sb[:], sig_o[:], tanh_c[:])

                # h2^T for the AllGather (and for BN, whose batch axis must be
                # the free axis). Transpose rounds straight to fp32r in PSUM
                # so the comm payload can be DMA'd from PSUM with no extra
                # cast on the critical path.
                pT = ptpool.tile([HL, B], MM_DT)
                nc.tensor.transpose(pT[:], h2_sb[:], ident_sb[:])
                h2T = wpool.tile([HL, B], MM_DT)
                nc.vector.tensor_copy(h2T[:], pT[:])

                if t < t_steps - 1:
                    cc_in = dpool.tile([HL, B], MM_DT, tag="cc_in")
                    r = HL // CCIN_DMA_SPLIT
                    cc_dma = None
                    for s in range(CCIN_DMA_SPLIT):
                        cc_dma = nc.gpsimd.dma_start(
                            out=cc_in[s * r : (s + 1) * r, :],
                            in_=h2T[s * r : (s + 1) * r, :],
                        )
                    cc_out = dpool.tile(
                        [NCORES * HL, B], MM_DT, tag="cc_out", addr_space="Shared"
                    )
                    nc.gpsimd.collective_compute(
                        "AllGather",
                        mybir.AluOpType.bypass,
                        replica_groups=rg,
                        ins=[cc_in[:]],
                        outs=[cc_out[:]],
                    )
                    # Keep the PE's activity monitor from re-throttling the
                    # clock during the AllGather gap: one warmer group fires
                    # at AG start (behind the comm-payload DMA), a second
                    # fires mid-AG behind a ~130KB delay DMA (~4us).
                    if N_WARM:
                        delay_sb = wpool.tile([128, 192], F32, tag="delay")
                        delay_dma = nc.scalar.dma_start(
                            out=delay_sb[:],
                            in_=xr_full[t * 128 : (t + 1) * 128, :],
                        )
                        add_dep_helper(
                            delay_dma.ins, cc_dma.ins, sync=True,
                            reason="delay DMA paces mid-AG PE warmers",
                        )
                        warm_ps = wmpool.tile([1, GL], F32, tag="warm")
                        for grp, n_grp in ((0, N_WARM // 2), (1, N_WARM // 2)):
                            first = None
                            for _ in range(n_grp):
                                wmm = nc.tensor.matmul(
                                    warm_ps[:],
                                    lhsT=x_r[:, 0, 0:1],
                                    rhs=w_r[:, 0, :],
                                    start=True,
                                    stop=True,
                                )
                                if first is None:
                                    gate = cc_dma if grp == 0 else delay_dma
                                    add_dep_helper(
                                        wmm.ins, gate.ins, sync=True,
                                        reason="PE warmer pacing",
                                    )
                                    first = wmm
                    hT_next = hpool.tile([128, KH, B], MM_DT)
                    step = KH // HT_DMA_SPLIT
                    qeng = [nc.sync, nc.scalar, nc.gpsimd, nc.sync,
                            nc.scalar, nc.gpsimd, nc.sync, nc.scalar]
                    for s in range(HT_DMA_SPLIT):
                        qeng[s % len(qeng)].dma_start(
                            out=hT_next[:, s * step : (s + 1) * step, :],
                            in_=cc_out[
                                s * step * 128 : (s + 1) * step * 128, :
                            ].rearrange("(k p) b -> p k b", p=128),
                        )
                    hT_cur = hT_next

                # BatchNorm (train mode, biased var over the full batch) +
                # temporal max-pool — off the critical path.
                stats6 = bnpool.tile([HL, 6], F32)
                nc.vector.bn_stats(stats6[:], h2T[:])
                mv = bnpool.tile([HL, 2], F32)
                nc.vector.bn_aggr(mv[:], stats6[:])
                std = bnpool.tile([HL, 1], F32)
                nc.scalar.activation(
                    std[:], mv[:, 1:2],
                    mybir.ActivationFunctionType.Sqrt,
                    bias=eps_sb[:],
                )
                rinv = bnpool.tile([HL, 1], F32)
                nc.vector.reciprocal(rinv[:], std[:])
                scl = bnpool.tile([HL, 1], F32)
                nc.vector.tensor_mul(scl[:], rinv[:], gam_sb[:])
                msc = bnpool.tile([HL, 1], F32)
                nc.vector.tensor_mul(msc[:], mv[:, 0:1], scl[:])
                shf = bnpool.tile([HL, 1], F32)
                nc.vector.tensor_sub(shf[:], bet_sb[:], msc[:])
                hn = bnpool.tile([HL, B], F32)
                nc.vector.tensor_scalar(
                    hn[:], h2T[:], scl[:], shf[:],
                    op0=mybir.AluOpType.mult, op1=mybir.AluOpType.add,
                )
                nc.vector.tensor_max(pooled_sb[:], pooled_sb[:], hn[:])

            nc.sync.dma_start(out=pooled_out[:], in_=pooled_sb[:])

    nc.compile()
    return nc


def prep_inputs(x, W_ih, W_hh, b_ih, b_hh, gamma, beta, t_steps=T):
    """Build the 8 per-core input maps (numpy only)."""
    x = np.asarray(x, np.float32)
    W_ih = np.asarray(W_ih, np.float32)
    W_hh = np.asarray(W_hh, np.float32)
    bias = np.asarray(b_ih, np.float32) + np.asarray(b_hh, np.float32)
    gamma = np.asarray(gamma, np.float32)
    beta = np.asarray(beta, np.float32)
    b_sz, t_sz = x.shape[0], x.shape[1]
    t_chunk = t_steps // NCORES

    # x pre-shuffle: xr[t, p, kk, b] = x[b, t, kk*128+p], row 383 = 1.0 (bias).
    xpad = np.zeros((t_sz, DP, b_sz), np.float32)
    xpad[:, :D, :] = x.transpose(1, 2, 0)
    xpad[:, DP - 1, :] = 1.0
    xr = np.ascontiguousarray(
        xpad.reshape(t_sz, KX, 128, b_sz).transpose(0, 2, 1, 3)
    ).reshape(t_sz, 128, KX * b_sz)

    ident = np.eye(b_sz, dtype=np.float32)
    in_maps = []
    for k in range(NCORES):
        idx = np.arange(k * HL, (k + 1) * HL)
        gate_rows = np.concatenate([g * H + idx for g in (0, 1, 3, 2)])  # i,f,o,g
        wT = np.zeros((KT * 128, GL), np.float32)
        wT[:H, :] = W_hh[gate_rows, :].T
        wT[H : H + D, :] = W_ih[gate_rows, :].T
        wT[H + DP - 1, :] = bias[gate_rows]
        w_tiles = np.ascontiguousarray(wT.reshape(KT, 128, GL).transpose(1, 0, 2))
        in_maps.append(
            {
                "w": w_tiles,
                "xc": np.ascontiguousarray(
                    xr[k * t_chunk : (k + 1) * t_chunk]
                ).reshape(t_chunk * 128, KX * b_sz),
                "gam": np.ascontiguousarray(gamma[idx][:, None]),
                "bet": np.ascontiguousarray(beta[idx][:, None]),
                "ident": ident,
            }
        )
    return in_maps


_NC_CACHE = {}


def run_cores(in_maps, t_steps=T, trace=False):
    global LAST_RESULTS
    if t_steps not in _NC_CACHE:
        _NC_CACHE[t_steps] = build_nc(t_steps)
    nc = _NC_CACHE[t_steps]
    res = bass_utils.run_bass_kernel_spmd(
        nc, in_maps, core_ids=list(range(NCORES)), trace=trace
    )
    LAST_RESULTS = res
    return res


def kernel(x, t, mask, W_ih, W_hh, b_ih, b_hh, gamma, beta, fc_w, fc_b):
    in_maps = prep_inputs(x, W_ih, W_hh, b_ih, b_hh, gamma, beta)
    trace = os.environ.get("LSTM_TRACE", "0") == "1"
    res = run_cores(in_maps, T, trace=trace)

    pooledT = np.stack([res.results[k]["pooled"] for k in range(NCORES)])
    pooled = pooledT.transpose(2, 0, 1).reshape(B, H)  # [b, core*128+p]
    pooled = pooled * np.asarray(mask, np.float32)
    z = pooled @ np.asarray(fc_w, np.float32)[0] + np.asarray(fc_b, np.float32)[0]
    tt = np.asarray(t, np.float32)
    loss = np.float32(np.mean(np.logaddexp(np.float32(0.0), z) - z * tt))
    return (loss, z.astype(np.float32))
